# revision 2
# baseline (speedup 1.0000x reference)
"""Binary conv (BN -> sign -> binarized 3x3 conv -> bias -> relu) on 8 TRN2 cores.

Strategy
--------
Data-parallel over batch: each of the 8 NeuronCores gets 8 of the 64 images.

  phase P (prologue):  load w, sign() -> bf16, PE-transpose each [co,ci] 128x128
                       block into lhsT layout [ci, co] per (tap, ci_chunk, co_chunk).
  phase A (stats):     stream x shard; DVE reduce_sum accumulates per-channel sums,
                       ScalarE Square+accum_out accumulates per-channel sum-of-squares.
                       One [128,4] fp32 AllReduce across the 8 cores; then
                       scale_c = gamma_c * rsqrt(var_c+eps), shift_c = beta_c - mean_c*scale_c.
  phase B (conv):      per image: ACT computes sign(scale*x + shift) -> bf16 into a
                       zero-padded [58x58] SBUF image; conv as 9 taps x 2 ci-chunks = 18
                       accumulating matmuls per [128co x 448px] PSUM tile (7 tiles of
                       8 rows per image per co-chunk); ACT fuses +bias and relu on the
                       PSUM evacuation; DMA out.

Because sign() outputs are exactly +-1 and bf16 represents +-1 exactly, and the PE
accumulates in fp32 (integer sums bounded by 2304), the conv arithmetic is exact.
"""

import os
import sys

import numpy as np

for _p in ("/opt/trn_rl_repo", "/root/.axon_site/_ro/trn_rl_repo"):
    if os.path.isdir(_p) and _p not in sys.path:
        sys.path.append(_p)

import concourse.bass as bass
import concourse.bacc as bacc
import concourse.tile as tile
from concourse import mybir
from concourse.bass_utils import run_bass_kernel_spmd
from concourse.masks import make_identity

AF = mybir.ActivationFunctionType
ALU = mybir.AluOpType
F32 = mybir.dt.float32
BF16 = mybir.dt.bfloat16

N_CORES = 8
N_IMG = 8          # images per core
C = 256            # channels (in == out)
H = W = 56
HW = H * W         # 3136
PW = W + 2         # 58 padded
EPS = 1e-5
N_TOTAL = 64 * HW  # BN reduction count over full batch
ROWS_PER_BLK = 8
N_BLK = H // ROWS_PER_BLK  # 7
BLK_FREE = ROWS_PER_BLK * W  # 448

_CACHE = {}


def _build_nc():
    nc = bacc.Bacc(None, target_bir_lowering=False, num_devices=N_CORES)

    x_d = nc.dram_tensor("x", [N_IMG, C, HW], F32, kind="ExternalInput")
    g_d = nc.dram_tensor("gamma", [C], F32, kind="ExternalInput")
    be_d = nc.dram_tensor("beta", [C], F32, kind="ExternalInput")
    w_d = nc.dram_tensor("w", [C, C * 9], F32, kind="ExternalInput")
    b_d = nc.dram_tensor("b", [C], F32, kind="ExternalInput")
    y_d = nc.dram_tensor("y", [N_IMG, C, HW], F32, kind="ExternalOutput")
    cc_in = nc.dram_tensor("cc_in", [128, 4], F32)
    cc_out = nc.dram_tensor("cc_out", [128, 4], F32, addr_space="Shared")

    with tile.TileContext(nc) as tc:
        with (
            tc.tile_pool(name="persist", bufs=1) as persist,
            tc.tile_pool(name="xin", bufs=3) as xin_pool,
            tc.tile_pool(name="outp", bufs=4) as out_pool,
            tc.tile_pool(name="vec", bufs=1) as vec_pool,
        ):
            # padded, binarized activations: [ci_part, ci_chunk, img, row, col]
            xpad = persist.tile([128, 2, N_IMG, PW, PW], BF16)
            # conv weights in lhsT layout: [ci_part, tap, ci_chunk, co_chunk, co]
            wt = persist.tile([128, 9, 2, 2, 128], BF16)

            # zero the one-pixel border of every padded image plane
            nc.vector.memset(xpad[:, :, :, 0, :], 0.0)
            nc.vector.memset(xpad[:, :, :, PW - 1, :], 0.0)
            nc.vector.memset(xpad[:, :, :, :, 0], 0.0)
            nc.vector.memset(xpad[:, :, :, :, PW - 1], 0.0)

            # per-channel vectors, [128, 2] = (partition, ci_chunk)
            gamma_sb = vec_pool.tile([128, 2], F32)
            beta_sb = vec_pool.tile([128, 2], F32)
            bias_sb = vec_pool.tile([128, 2], F32)
            nc.sync.dma_start(gamma_sb, g_d.rearrange("(c p) -> p c", p=128))
            nc.sync.dma_start(beta_sb, be_d.rearrange("(c p) -> p c", p=128))
            nc.sync.dma_start(bias_sb, b_d.rearrange("(c p) -> p c", p=128))

            # ---------------- phase P: weights ----------------
            with (
                tc.tile_pool(name="wpro", bufs=2) as wpro,
                tc.tile_pool(name="wps", bufs=2, space="PSUM") as wps,
            ):
                ident = wpro.tile([128, 128], BF16)
                make_identity(nc, ident)
                ws = wpro.tile([128, 2, C * 9], BF16)
                for o in range(2):
                    wf = wpro.tile([128, C * 9], F32)
                    nc.sync.dma_start(wf, w_d[o * 128 : (o + 1) * 128, :])
                    nc.scalar.activation(ws[:, o, :], wf, AF.Sign)
                ws_r = ws.rearrange("p o (ci tap) -> p o ci tap", tap=9)
                for t in range(9):
                    for c in range(2):
                        for o in range(2):
                            pw = wps.tile([128, 128], BF16)
                            nc.tensor.transpose(
                                pw, ws_r[:, o, c * 128 : (c + 1) * 128, t], ident
                            )
                            nc.vector.tensor_copy(wt[:, t, c, o, :], pw)

            # ---------------- phase A: BN stats ----------------
            sums = vec_pool.tile([128, 16], F32)    # per (chunk*8 + img)
            sumsq = vec_pool.tile([128, 16], F32)
            with tc.tile_pool(name="trash", bufs=2) as trash_pool:
                for c in range(2):
                    for n in range(N_IMG):
                        xt = xin_pool.tile([128, HW], F32)
                        nc.sync.dma_start(xt, x_d[n, c * 128 : (c + 1) * 128, :])
                        k = c * 8 + n
                        nc.vector.reduce_sum(
                            sums[:, k : k + 1], xt, axis=mybir.AxisListType.X
                        )
                        tr = trash_pool.tile([128, HW], F32)
                        nc.scalar.activation(
                            tr, xt, AF.Square, accum_out=sumsq[:, k : k + 1]
                        )

            cc_sb = vec_pool.tile([128, 4], F32)
            for c in range(2):
                nc.vector.reduce_sum(
                    cc_sb[:, c : c + 1], sums[:, c * 8 : (c + 1) * 8],
                    axis=mybir.AxisListType.X,
                )
                nc.vector.reduce_sum(
                    cc_sb[:, 2 + c : 3 + c], sumsq[:, c * 8 : (c + 1) * 8],
                    axis=mybir.AxisListType.X,
                )
            nc.sync.dma_start(cc_in[:], cc_sb)
            nc.gpsimd.collective_compute(
                "AllReduce",
                ALU.add,
                replica_groups=[list(range(N_CORES))],
                ins=[cc_in[:]],
                outs=[cc_out[:]],
            )
            gl = vec_pool.tile([128, 4], F32)
            nc.sync.dma_start(gl, cc_out[:])

            # mean/var -> scale/shift
            mean = vec_pool.tile([128, 2], F32)
            var = vec_pool.tile([128, 2], F32)
            std = vec_pool.tile([128, 2], F32)
            rstd = vec_pool.tile([128, 2], F32)
            scl = vec_pool.tile([128, 2], F32)
            sh = vec_pool.tile([128, 2], F32)
            nc.vector.tensor_scalar_mul(mean, gl[:, 0:2], 1.0 / N_TOTAL)
            nc.vector.tensor_scalar_mul(var, gl[:, 2:4], 1.0 / N_TOTAL)  # E[x^2]
            nc.vector.tensor_tensor(
                std, mean, mean, op=ALU.mult
            )  # std <- mean^2 (scratch)
            nc.vector.tensor_sub(var, var, std)  # var = E[x^2] - mean^2
            eps_sb = vec_pool.tile([128, 1], F32)
            nc.vector.memset(eps_sb, EPS)
            nc.scalar.activation(std, var, AF.Sqrt, bias=eps_sb[:])
            nc.vector.reciprocal(rstd, std)
            nc.vector.tensor_mul(scl, gamma_sb, rstd)
            nc.vector.tensor_mul(sh, mean, scl)
            nc.vector.tensor_sub(sh, beta_sb, sh)  # shift = beta - mean*scale

            # ---------------- phase B: sign + conv ----------------
            with tc.tile_pool(name="cps", bufs=8, space="PSUM") as cps:
                for n in range(N_IMG):
                    for c in range(2):
                        xt = xin_pool.tile([128, HW], F32)
                        nc.sync.dma_start(xt, x_d[n, c * 128 : (c + 1) * 128, :])
                        nc.scalar.activation(
                            xpad[:, c, n, 1 : H + 1, 1 : W + 1],
                            xt.rearrange("p (h w) -> p h w", w=W),
                            AF.Sign,
                            bias=sh[:, c : c + 1],
                            scale=scl[:, c : c + 1],
                        )
                    for o in range(2):
                        for bi in range(N_BLK):
                            ps = cps.tile([128, BLK_FREE], F32)
                            r0 = bi * ROWS_PER_BLK
                            for t in range(9):
                                ky, kx = divmod(t, 3)
                                for c in range(2):
                                    nc.tensor.matmul(
                                        ps,
                                        wt[:, t, c, o, :],
                                        xpad[
                                            :, c, n,
                                            r0 + ky : r0 + ky + ROWS_PER_BLK,
                                            kx : kx + W,
                                        ],
                                        start=(t == 0 and c == 0),
                                        stop=(t == 8 and c == 1),
                                    )
                            ob = out_pool.tile([128, BLK_FREE], F32)
                            nc.scalar.activation(
                                ob, ps, AF.Relu, bias=bias_sb[:, o : o + 1]
                            )
                            nc.sync.dma_start(
                                y_d[
                                    n, o * 128 : (o + 1) * 128,
                                    bi * BLK_FREE : (bi + 1) * BLK_FREE,
                                ],
                                ob,
                            )

    nc.finalize()
    return nc


def get_nc():
    if "nc" not in _CACHE:
        _CACHE["nc"] = _build_nc()
    return _CACHE["nc"]


def run(x, gamma, beta, w, b, trace=False):
    x = np.ascontiguousarray(np.asarray(x, dtype=np.float32))
    gamma = np.ascontiguousarray(np.asarray(gamma, dtype=np.float32))
    beta = np.ascontiguousarray(np.asarray(beta, dtype=np.float32))
    w = np.ascontiguousarray(np.asarray(w, dtype=np.float32)).reshape(C, C * 9)
    b = np.ascontiguousarray(np.asarray(b, dtype=np.float32))

    nc = get_nc()
    in_maps = []
    for i in range(N_CORES):
        in_maps.append(
            {
                "x": np.ascontiguousarray(
                    x[i * N_IMG : (i + 1) * N_IMG].reshape(N_IMG, C, HW)
                ),
                "gamma": gamma,
                "beta": beta,
                "w": w,
                "b": b,
            }
        )
    res = run_bass_kernel_spmd(
        nc, in_maps, list(range(N_CORES)), trace=trace
    )
    y = np.concatenate(
        [r["y"].reshape(N_IMG, C, H, W) for r in res.results], axis=0
    )
    return y.astype(np.float32), res


def kernel(x, gamma, beta, w, b):
    y, _ = run(x, gamma, beta, w, b, trace=False)
    return y


# revision 4
# speedup vs baseline: 1.4882x; 1.4882x over previous
"""Binary conv (BN -> sign -> binarized 3x3 conv -> bias -> relu) on 8 TRN2 cores.

Strategy
--------
Data-parallel over batch: each of the 8 NeuronCores gets 8 of the 64 images.

  phase P (prologue):  load w, sign() -> bf16, PE-transpose each [co,ci] 128x128
                       block into lhsT layout, store as fp8e4 [ci, 2, co] pairs.
  phase A (stats):     stream x shard; DVE reduce_sum accumulates per-channel sums,
                       ScalarE Square+accum_out accumulates per-channel sum-of-squares.
                       One [128,4] fp32 AllReduce across the 8 cores; then
                       scale_c = gamma_c * rsqrt(var_c+eps), shift_c = beta_c - mean_c*scale_c.
  phase B (conv):      per image: ACT computes sign(scale*x + shift) -> fp8e4 into a
                       zero-padded flat [58*58] SBUF plane (1 guard elem on each side);
                       conv as 9 taps x fp8 DoubleRow matmuls (contracting all 256 ci
                       at once) into [128co x 464px] PSUM tiles over contiguous 8-row
                       windows (the 2 wrap columns are computed and discarded);
                       DVE fuses +bias and relu on the PSUM evacuation; DMA out.

sign() outputs +-1 exactly representable in fp8e4, PE accumulates in fp32
(integer sums bounded by 2304), so the conv arithmetic is exact.
"""

import os
import sys

import numpy as np

for _p in ("/opt/trn_rl_repo", "/root/.axon_site/_ro/trn_rl_repo"):
    if os.path.isdir(_p) and _p not in sys.path:
        sys.path.append(_p)

import concourse.bass as bass
import concourse.bacc as bacc
import concourse.tile as tile
from concourse import mybir
from concourse.bass_utils import run_bass_kernel_spmd
from concourse.masks import make_identity

AF = mybir.ActivationFunctionType
ALU = mybir.AluOpType
F32 = mybir.dt.float32
BF16 = mybir.dt.bfloat16
FP8 = mybir.dt.float8e4

N_CORES = 8
N_IMG = 8          # images per core
C = 256            # channels (in == out)
H = W = 56
HW = H * W         # 3136
PW = W + 2         # 58 padded
PLANE = PW * PW    # 3364
PLANE_G = PLANE + 2  # plus 1 guard element on each side
EPS = 1e-5
N_TOTAL = 64 * HW  # BN reduction count over full batch
ROWS_PER_BLK = 8
N_BLK = H // ROWS_PER_BLK        # 7
BLK_FREE = ROWS_PER_BLK * PW     # 464 (incl. 2 wrap columns/row)
OUT_FREE = ROWS_PER_BLK * W      # 448 valid outputs

_CACHE = {}


def _build_nc():
    nc = bacc.Bacc(None, target_bir_lowering=False, num_devices=N_CORES)

    x_d = nc.dram_tensor("x", [N_IMG, C, HW], F32, kind="ExternalInput")
    g_d = nc.dram_tensor("gamma", [C], F32, kind="ExternalInput")
    be_d = nc.dram_tensor("beta", [C], F32, kind="ExternalInput")
    w_d = nc.dram_tensor("w", [C, C * 9], F32, kind="ExternalInput")
    b_d = nc.dram_tensor("b", [C], F32, kind="ExternalInput")
    y_d = nc.dram_tensor("y", [N_IMG, C, HW], F32, kind="ExternalOutput")
    cc_in = nc.dram_tensor("cc_in", [128, 4], F32)
    cc_out = nc.dram_tensor("cc_out", [128, 4], F32, addr_space="Shared")

    with tile.TileContext(nc) as tc:
        with (
            tc.tile_pool(name="persist", bufs=1) as persist,
            tc.tile_pool(name="xin", bufs=5) as xin_pool,
            tc.tile_pool(name="outp", bufs=4) as out_pool,
            tc.tile_pool(name="vec", bufs=1) as vec_pool,
        ):
            # padded+binarized activations: [ci_part, ci_pair(j), img, guarded flat plane]
            xpad = persist.tile([128, 2, N_IMG, PLANE_G], FP8)
            # conv weights, fp8 DoubleRow lhsT layout: [ci_part, tap, co_chunk, j, co]
            wt = persist.tile([128, 9, 2, 2, 128], FP8)

            # zero borders (rows 0/57, cols 0/57 of each plane) + guard elements
            # (plane data starts at flat offset 1; offset 0 / PLANE+1 are guards)
            xrow = xpad[:, :, :, 1 : 1 + PLANE].rearrange(
                "p j n (r c) -> p j n r c", c=PW
            )
            nc.vector.memset(xrow[:, :, :, 0, :], 0.0)
            nc.vector.memset(xrow[:, :, :, PW - 1, :], 0.0)
            nc.vector.memset(xrow[:, :, :, :, 0], 0.0)
            nc.vector.memset(xrow[:, :, :, :, PW - 1], 0.0)
            nc.vector.memset(xpad[:, :, :, 0:1], 0.0)
            nc.vector.memset(xpad[:, :, :, PLANE + 1 : PLANE_G], 0.0)

            # per-channel vectors, [128, 2] = (partition, ci_chunk)
            gamma_sb = vec_pool.tile([128, 2], F32)
            beta_sb = vec_pool.tile([128, 2], F32)
            bias_sb = vec_pool.tile([128, 2], F32)
            nc.sync.dma_start(gamma_sb, g_d.rearrange("(c p) -> p c", p=128))
            nc.sync.dma_start(beta_sb, be_d.rearrange("(c p) -> p c", p=128))
            nc.sync.dma_start(bias_sb, b_d.rearrange("(c p) -> p c", p=128))

            # ---------------- phase P: weights ----------------
            with (
                tc.tile_pool(name="wpro", bufs=2) as wpro,
                tc.tile_pool(name="wps", bufs=2, space="PSUM") as wps,
            ):
                ident = wpro.tile([128, 128], BF16)
                make_identity(nc, ident)
                ws = wpro.tile([128, 2, C * 9], BF16)
                for o in range(2):
                    wf = wpro.tile([128, C * 9], F32)
                    nc.sync.dma_start(wf, w_d[o * 128 : (o + 1) * 128, :])
                    nc.scalar.activation(ws[:, o, :], wf, AF.Sign)
                ws_r = ws.rearrange("p o (ci tap) -> p o ci tap", tap=9)
                for t in range(9):
                    for c in range(2):
                        for o in range(2):
                            pw = wps.tile([128, 128], BF16)
                            nc.tensor.transpose(
                                pw, ws_r[:, o, c * 128 : (c + 1) * 128, t], ident
                            )
                            nc.vector.tensor_copy(wt[:, t, o, c, :], pw)

            # ---------------- phase A: BN stats ----------------
            sums = vec_pool.tile([128, 16], F32)    # per (chunk*8 + img)
            sumsq = vec_pool.tile([128, 16], F32)
            with tc.tile_pool(name="trash", bufs=2) as trash_pool:
                for c in range(2):
                    for n in range(N_IMG):
                        xt = xin_pool.tile([128, HW], F32)
                        nc.sync.dma_start(xt, x_d[n, c * 128 : (c + 1) * 128, :])
                        k = c * 8 + n
                        nc.vector.reduce_sum(
                            sums[:, k : k + 1], xt, axis=mybir.AxisListType.X
                        )
                        tr = trash_pool.tile([128, HW], F32)
                        nc.scalar.activation(
                            tr, xt, AF.Square, accum_out=sumsq[:, k : k + 1]
                        )

            cc_sb = vec_pool.tile([128, 4], F32)
            for c in range(2):
                nc.vector.reduce_sum(
                    cc_sb[:, c : c + 1], sums[:, c * 8 : (c + 1) * 8],
                    axis=mybir.AxisListType.X,
                )
                nc.vector.reduce_sum(
                    cc_sb[:, 2 + c : 3 + c], sumsq[:, c * 8 : (c + 1) * 8],
                    axis=mybir.AxisListType.X,
                )
            nc.sync.dma_start(cc_in[:], cc_sb)
            nc.gpsimd.collective_compute(
                "AllReduce",
                ALU.add,
                replica_groups=[list(range(N_CORES))],
                ins=[cc_in[:]],
                outs=[cc_out[:]],
            )
            gl = vec_pool.tile([128, 4], F32)
            nc.sync.dma_start(gl, cc_out[:])

            # mean/var -> scale/shift
            mean = vec_pool.tile([128, 2], F32)
            var = vec_pool.tile([128, 2], F32)
            std = vec_pool.tile([128, 2], F32)
            rstd = vec_pool.tile([128, 2], F32)
            scl = vec_pool.tile([128, 2], F32)
            sh = vec_pool.tile([128, 2], F32)
            nc.vector.tensor_scalar_mul(mean, gl[:, 0:2], 1.0 / N_TOTAL)
            nc.vector.tensor_scalar_mul(var, gl[:, 2:4], 1.0 / N_TOTAL)  # E[x^2]
            nc.vector.tensor_tensor(
                std, mean, mean, op=ALU.mult
            )  # std <- mean^2 (scratch)
            nc.vector.tensor_sub(var, var, std)  # var = E[x^2] - mean^2
            eps_sb = vec_pool.tile([128, 1], F32)
            nc.vector.memset(eps_sb, EPS)
            nc.scalar.activation(std, var, AF.Sqrt, bias=eps_sb[:])
            nc.vector.reciprocal(rstd, std)
            nc.vector.tensor_mul(scl, gamma_sb, rstd)
            nc.vector.tensor_mul(sh, mean, scl)
            nc.vector.tensor_sub(sh, beta_sb, sh)  # shift = beta - mean*scale

            # ---------------- phase B: sign + conv ----------------
            with tc.tile_pool(name="cps", bufs=8, space="PSUM") as cps:
                for n in range(N_IMG):
                    for c in range(2):
                        xt = xin_pool.tile([128, HW], F32)
                        nc.sync.dma_start(xt, x_d[n, c * 128 : (c + 1) * 128, :])
                        nc.scalar.activation(
                            xrow[:, c, n, 1 : H + 1, 1 : W + 1],
                            xt.rearrange("p (h w) -> p h w", w=W),
                            AF.Sign,
                            bias=sh[:, c : c + 1],
                            scale=scl[:, c : c + 1],
                        )
                    for o in range(2):
                        for bi in range(N_BLK):
                            ps = cps.tile([128, BLK_FREE], F32)
                            r0 = bi * ROWS_PER_BLK
                            for t in range(9):
                                ky, kx = divmod(t, 3)
                                base = 1 + (r0 + ky) * PW + (kx - 1)
                                nc.tensor.matmul(
                                    ps,
                                    wt[:, t, o],
                                    xpad[:, :, n, base : base + BLK_FREE],
                                    start=(t == 0),
                                    stop=(t == 8),
                                    perf_mode=mybir.MatmulPerfMode.DoubleRow,
                                )
                            ob = out_pool.tile([128, OUT_FREE], F32)
                            # relu(psum + bias): (x + b) then max(.., 0) on DVE,
                            # dropping the 2 wrap columns of each row
                            nc.vector.tensor_scalar(
                                out=ob,
                                in0=ps.rearrange("p (r c) -> p r c", c=PW)[
                                    :, :, 1 : W + 1
                                ],
                                scalar1=bias_sb[:, o : o + 1],
                                scalar2=0.0,
                                op0=ALU.add,
                                op1=ALU.max,
                            )
                            nc.sync.dma_start(
                                y_d[
                                    n, o * 128 : (o + 1) * 128,
                                    bi * OUT_FREE : (bi + 1) * OUT_FREE,
                                ],
                                ob,
                            )

    nc.finalize()
    return nc


def get_nc():
    if "nc" not in _CACHE:
        _CACHE["nc"] = _build_nc()
    return _CACHE["nc"]


def run(x, gamma, beta, w, b, trace=False, trace_cores=None):
    x = np.ascontiguousarray(np.asarray(x, dtype=np.float32))
    gamma = np.ascontiguousarray(np.asarray(gamma, dtype=np.float32))
    beta = np.ascontiguousarray(np.asarray(beta, dtype=np.float32))
    w = np.ascontiguousarray(np.asarray(w, dtype=np.float32)).reshape(C, C * 9)
    b = np.ascontiguousarray(np.asarray(b, dtype=np.float32))

    nc = get_nc()
    in_maps = []
    for i in range(N_CORES):
        in_maps.append(
            {
                "x": np.ascontiguousarray(
                    x[i * N_IMG : (i + 1) * N_IMG].reshape(N_IMG, C, HW)
                ),
                "gamma": gamma,
                "beta": beta,
                "w": w,
                "b": b,
            }
        )
    res = run_bass_kernel_spmd(
        nc, in_maps, list(range(N_CORES)), trace=trace, trace_cores=trace_cores
    )
    y = np.concatenate(
        [r["y"].reshape(N_IMG, C, H, W) for r in res.results], axis=0
    )
    return y.astype(np.float32), res


def kernel(x, gamma, beta, w, b):
    y, _ = run(x, gamma, beta, w, b, trace=False)
    return y


# revision 8
# speedup vs baseline: 1.5793x; 1.0613x over previous
"""Binary conv (BN -> sign -> binarized 3x3 conv -> bias -> relu) on 8 TRN2 cores.

Strategy
--------
Data-parallel over batch: each of the 8 NeuronCores gets 8 of the 64 images.

  phase P (prologue):  load w, sign() -> bf16, PE-transpose each [co,ci] 128x128
                       block into lhsT layout, store as fp8e4 [ci, 2, co] pairs.
  phase A (stats):     stream x shard; DVE reduce_sum accumulates per-channel sums,
                       ScalarE Square+accum_out accumulates per-channel sum-of-squares.
                       One [128,4] fp32 AllReduce across the 8 cores; then
                       scale_c = gamma_c * rsqrt(var_c+eps), shift_c = beta_c - mean_c*scale_c.
  phase B (conv):      per image: ACT computes sign(scale*x + shift) -> fp8e4 into a
                       zero-padded flat [58*58] SBUF plane (1 guard elem on each side);
                       conv as 9 taps x fp8 DoubleRow matmuls (contracting all 256 ci
                       at once) into [128co x 464px] PSUM tiles over contiguous 8-row
                       windows (the 2 wrap columns are computed and discarded);
                       DVE fuses +bias and relu on the PSUM evacuation; DMA out.

sign() outputs +-1 exactly representable in fp8e4, PE accumulates in fp32
(integer sums bounded by 2304), so the conv arithmetic is exact.
"""

import os
import sys

import numpy as np

for _p in ("/opt/trn_rl_repo", "/root/.axon_site/_ro/trn_rl_repo"):
    if os.path.isdir(_p) and _p not in sys.path:
        sys.path.append(_p)

import concourse.bass as bass
import concourse.bacc as bacc
import concourse.tile as tile
from concourse import mybir
from concourse.bass_utils import run_bass_kernel_spmd
from concourse.masks import make_identity

AF = mybir.ActivationFunctionType
ALU = mybir.AluOpType
F32 = mybir.dt.float32
BF16 = mybir.dt.bfloat16
FP8 = mybir.dt.float8e4

N_CORES = 8
N_IMG = 8          # images per core
C = 256            # channels (in == out)
H = W = 56
HW = H * W         # 3136
PW = W + 2         # 58 padded
PLANE = PW * PW    # 3364
# plane data at offset 1 (1 guard elem before, guards/pad after); padded so the
# DoubleRow pair stride (N_IMG//2 * PLANE_G fp8 elements) is a multiple of 16
PLANE_G = PLANE + 4  # 3368
EPS = 1e-5
N_TOTAL = 64 * HW  # BN reduction count over full batch
ROWS_PER_BLK = 8
N_BLK = H // ROWS_PER_BLK        # 7
BLK_FREE = ROWS_PER_BLK * PW     # 464 (incl. 2 wrap columns/row)
OUT_FREE = ROWS_PER_BLK * W      # 448 valid outputs

_CACHE = {}


def _build_nc():
    nc = bacc.Bacc(None, target_bir_lowering=False, num_devices=N_CORES)

    x_d = nc.dram_tensor("x", [N_IMG, C, HW], F32, kind="ExternalInput")
    g_d = nc.dram_tensor("gamma", [C], F32, kind="ExternalInput")
    be_d = nc.dram_tensor("beta", [C], F32, kind="ExternalInput")
    w_d = nc.dram_tensor("w", [C, C * 9], F32, kind="ExternalInput")
    b_d = nc.dram_tensor("b", [C], F32, kind="ExternalInput")
    y_d = nc.dram_tensor("y", [N_IMG, C, HW], F32, kind="ExternalOutput")
    cc_in = nc.dram_tensor("cc_in", [128, 4], F32)
    cc_out = nc.dram_tensor("cc_out", [128, 4], F32, addr_space="Shared")

    with tile.TileContext(nc) as tc:
        with (
            tc.tile_pool(name="persist", bufs=1) as persist,
            tc.tile_pool(name="xin", bufs=5) as xin_pool,
            tc.tile_pool(name="outp", bufs=4) as out_pool,
            tc.tile_pool(name="vec", bufs=1) as vec_pool,
        ):
            # padded+binarized activations, split by image parity so sign()
            # writes for image n+1 don't WAR-serialize against conv reads of
            # image n: [ci_part, ci_pair(j), img//2, guarded flat plane]
            xpadA = persist.tile([128, 2, N_IMG // 2, PLANE_G], FP8)
            xpadB = persist.tile([128, 2, N_IMG // 2, PLANE_G], FP8)
            xpads = [xpadA, xpadB]
            # conv weights, fp8 DoubleRow lhsT layout: [ci_part, tap, co_chunk, j, co]
            wt = persist.tile([128, 9, 2, 2, 128], FP8)

            # ---------------- phase A: BN stats (emitted first so the x
            # stream starts immediately) ----------------
            sums = vec_pool.tile([128, 16], F32)    # per (chunk*8 + img)
            sumsq = vec_pool.tile([128, 16], F32)
            N_CHUNK_DMA = 4  # sub-DMAs per tile -> first tile lands early
            with tc.tile_pool(name="trash", bufs=2) as trash_pool:
                for c in range(2):
                    for n in range(N_IMG):
                        xt = xin_pool.tile([128, HW], F32)
                        step = HW // N_CHUNK_DMA
                        for q in range(N_CHUNK_DMA):
                            nc.sync.dma_start(
                                xt[:, q * step : (q + 1) * step],
                                x_d[
                                    n, c * 128 : (c + 1) * 128,
                                    q * step : (q + 1) * step,
                                ],
                            )
                        k = c * 8 + n
                        nc.vector.reduce_sum(
                            sums[:, k : k + 1], xt, axis=mybir.AxisListType.X
                        )
                        tr = trash_pool.tile([128, HW], F32)
                        nc.scalar.activation(
                            tr, xt, AF.Square, accum_out=sumsq[:, k : k + 1]
                        )

            # zero borders (rows 0/57, cols 0/57 of each plane) + guard elements
            # (plane data starts at flat offset 1; offset 0 / PLANE+1 are guards)
            xrows = []
            for xp in xpads:
                xrow = xp[:, :, :, 1 : 1 + PLANE].rearrange(
                    "p j n (r c) -> p j n r c", c=PW
                )
                xrows.append(xrow)
                nc.vector.memset(xrow[:, :, :, 0, :], 0.0)
                nc.vector.memset(xrow[:, :, :, PW - 1, :], 0.0)
                nc.vector.memset(xrow[:, :, :, :, 0], 0.0)
                nc.vector.memset(xrow[:, :, :, :, PW - 1], 0.0)
                nc.vector.memset(xp[:, :, :, 0:1], 0.0)
                nc.vector.memset(xp[:, :, :, PLANE + 1 : PLANE_G], 0.0)

            # per-channel vectors, [128, 2] = (partition, ci_chunk)
            gamma_sb = vec_pool.tile([128, 2], F32)
            beta_sb = vec_pool.tile([128, 2], F32)
            bias_sb = vec_pool.tile([128, 2], F32)
            nc.sync.dma_start(gamma_sb, g_d.rearrange("(c p) -> p c", p=128))
            nc.sync.dma_start(beta_sb, be_d.rearrange("(c p) -> p c", p=128))
            nc.sync.dma_start(bias_sb, b_d.rearrange("(c p) -> p c", p=128))

            # ---------------- phase P: weights ----------------
            with (
                tc.tile_pool(name="wpro", bufs=2) as wpro,
                tc.tile_pool(name="wps", bufs=2, space="PSUM") as wps,
            ):
                ident = wpro.tile([128, 128], BF16)
                make_identity(nc, ident)
                ws = wpro.tile([128, 2, C * 9], BF16)
                for o in range(2):
                    wf = wpro.tile([128, C * 9], F32)
                    nc.sync.dma_start(wf, w_d[o * 128 : (o + 1) * 128, :])
                    nc.scalar.activation(ws[:, o, :], wf, AF.Sign)
                ws_r = ws.rearrange("p o (ci tap) -> p o ci tap", tap=9)
                for t in range(9):
                    for c in range(2):
                        for o in range(2):
                            pw = wps.tile([128, 128], BF16)
                            nc.tensor.transpose(
                                pw, ws_r[:, o, c * 128 : (c + 1) * 128, t], ident
                            )
                            nc.vector.tensor_copy(wt[:, t, o, c, :], pw)

            cc_sb = vec_pool.tile([128, 4], F32)
            for c in range(2):
                nc.vector.reduce_sum(
                    cc_sb[:, c : c + 1], sums[:, c * 8 : (c + 1) * 8],
                    axis=mybir.AxisListType.X,
                )
                nc.vector.reduce_sum(
                    cc_sb[:, 2 + c : 3 + c], sumsq[:, c * 8 : (c + 1) * 8],
                    axis=mybir.AxisListType.X,
                )
            nc.sync.dma_start(cc_in[:], cc_sb)
            nc.gpsimd.collective_compute(
                "AllReduce",
                ALU.add,
                replica_groups=[list(range(N_CORES))],
                ins=[cc_in[:]],
                outs=[cc_out[:]],
            )
            gl = vec_pool.tile([128, 4], F32)
            nc.sync.dma_start(gl, cc_out[:])

            # mean/var -> scale/shift
            mean = vec_pool.tile([128, 2], F32)
            var = vec_pool.tile([128, 2], F32)
            std = vec_pool.tile([128, 2], F32)
            rstd = vec_pool.tile([128, 2], F32)
            scl = vec_pool.tile([128, 2], F32)
            sh = vec_pool.tile([128, 2], F32)
            nc.vector.tensor_scalar_mul(mean, gl[:, 0:2], 1.0 / N_TOTAL)
            nc.vector.tensor_scalar_mul(var, gl[:, 2:4], 1.0 / N_TOTAL)  # E[x^2]
            nc.vector.tensor_tensor(
                std, mean, mean, op=ALU.mult
            )  # std <- mean^2 (scratch)
            nc.vector.tensor_sub(var, var, std)  # var = E[x^2] - mean^2
            eps_sb = vec_pool.tile([128, 1], F32)
            nc.vector.memset(eps_sb, EPS)
            nc.scalar.activation(std, var, AF.Sqrt, bias=eps_sb[:])
            nc.vector.reciprocal(rstd, std)
            nc.vector.tensor_mul(scl, gamma_sb, rstd)
            nc.vector.tensor_mul(sh, mean, scl)
            nc.vector.tensor_sub(sh, beta_sb, sh)  # shift = beta - mean*scale

            # ---------------- phase B: sign + conv ----------------
            with tc.tile_pool(name="cps", bufs=8, space="PSUM") as cps:
                for n in range(N_IMG):
                    xp = xpads[n % 2]
                    slot = n // 2
                    for c in range(2):
                        xt = xin_pool.tile([128, HW], F32)
                        nc.sync.dma_start(xt, x_d[n, c * 128 : (c + 1) * 128, :])
                        nc.scalar.activation(
                            xrows[n % 2][:, c, slot, 1 : H + 1, 1 : W + 1],
                            xt.rearrange("p (h w) -> p h w", w=W),
                            AF.Sign,
                            bias=sh[:, c : c + 1],
                            scale=scl[:, c : c + 1],
                        )
                    for o in range(2):
                        for bi in range(N_BLK):
                            ps = cps.tile([128, BLK_FREE], F32)
                            r0 = bi * ROWS_PER_BLK
                            for t in range(9):
                                ky, kx = divmod(t, 3)
                                base = 1 + (r0 + ky) * PW + (kx - 1)
                                nc.tensor.matmul(
                                    ps,
                                    wt[:, t, o],
                                    xp[:, :, slot, base : base + BLK_FREE],
                                    start=(t == 0),
                                    stop=(t == 8),
                                    perf_mode=mybir.MatmulPerfMode.DoubleRow,
                                )
                            ob = out_pool.tile([128, OUT_FREE], F32)
                            # relu(psum + bias): (x + b) then max(.., 0) on DVE,
                            # dropping the 2 wrap columns of each row
                            nc.vector.tensor_scalar(
                                out=ob,
                                in0=ps.rearrange("p (r c) -> p r c", c=PW)[
                                    :, :, 1 : W + 1
                                ],
                                scalar1=bias_sb[:, o : o + 1],
                                scalar2=0.0,
                                op0=ALU.add,
                                op1=ALU.max,
                            )
                            nc.sync.dma_start(
                                y_d[
                                    n, o * 128 : (o + 1) * 128,
                                    bi * OUT_FREE : (bi + 1) * OUT_FREE,
                                ],
                                ob,
                            )

    nc.finalize()
    return nc


def get_nc():
    if "nc" not in _CACHE:
        _CACHE["nc"] = _build_nc()
    return _CACHE["nc"]


def run(x, gamma, beta, w, b, trace=False, trace_cores=None):
    x = np.ascontiguousarray(np.asarray(x, dtype=np.float32))
    gamma = np.ascontiguousarray(np.asarray(gamma, dtype=np.float32))
    beta = np.ascontiguousarray(np.asarray(beta, dtype=np.float32))
    w = np.ascontiguousarray(np.asarray(w, dtype=np.float32)).reshape(C, C * 9)
    b = np.ascontiguousarray(np.asarray(b, dtype=np.float32))

    nc = get_nc()
    in_maps = []
    for i in range(N_CORES):
        in_maps.append(
            {
                "x": np.ascontiguousarray(
                    x[i * N_IMG : (i + 1) * N_IMG].reshape(N_IMG, C, HW)
                ),
                "gamma": gamma,
                "beta": beta,
                "w": w,
                "b": b,
            }
        )
    res = run_bass_kernel_spmd(
        nc, in_maps, list(range(N_CORES)), trace=trace, trace_cores=trace_cores
    )
    y = np.concatenate(
        [r["y"].reshape(N_IMG, C, H, W) for r in res.results], axis=0
    )
    return y.astype(np.float32), res


def kernel(x, gamma, beta, w, b):
    y, _ = run(x, gamma, beta, w, b, trace=False)
    return y
